# revision 1
# baseline (speedup 1.0000x reference)
"""KNN graph kernel (DenseDilatedKnnGraph) for Trainium2, 8 NeuronCores.

Problem: x [2, 192, 8192, 1] fp32 -> edge_index [2, 2, 8192, 9] int32.
reference: L2-normalize x along C, pairwise sq-dists over N, top-9 (k=9,
dilation=1) nearest neighbors (indices), stacked with center indices.

Math used here: for normalized points, ranking by -dist == ranking by
cosine = Xn^T Xn. The nearest neighbor is always the point itself
(cos=1 >> all others for this data), so the device computes the top-8
of the Gram matrix with the self-column masked out; the host prepends
the self index.

Sharding: 8 cores = 2 batches x 4 query-row-blocks of 2048. Each core
gets the full batch slice with its columns ROTATED so its own query
block sits at columns 0..2047 (keeps the SPMD program identical across
cores: the self-match diagonal is at a static position). Host maps
returned neighbor indices back by adding the rotation offset mod N.

Per core device pipeline (MODE="fp16x3"):
  1. Stream x in 1024-col chunks: squares (DVE), B-channel squares
     folded into the A rows, one K=128 ones-matmul -> norms^2, sqrt
     (ACT); reciprocal in a [128, 64] transposed layout (DVE, DRAM
     bounce), interleaved per 2048-col quarter.
  2. Build fp16 split of the normalized points (1/norm partition-
     broadcast by step-0 DMA): xn = h + l/32 + O(2^-24) with
     h = fp16(xn), l5 = fp16((xn-h)*32); weight-side scaled copies
     w2 = h/32, w3 = l5/32 for the query columns. PE computes fp16
     subnormals exactly, so this is fp32-grade.
  3. For each of 16 query row-tiles [128 x 8192]: Gram = h[t].h +
     w2[t].l5 + w3[t].h (6 fp16 passes per 512-col chunk, power-of-two
     scales cancel exactly), evacuate PSUM->SBUF (ACT), add -20 on the
     self diagonal, then per column HALF: DVE max (top-8) + max_index
     (jax top_k tie semantics). Host merges the 16 candidates by
     (-value, stable position) = exact jax tie order.
"""

import numpy as np

B = 2
C = 192
N = 8192
NCORES = 8
RBLK = N // 4  # 2048 query rows per core
CHUNK = 512
NCHUNK = N // CHUNK  # 16
NT = RBLK // 128  # 16 row tiles per core
NEG = -20.0

_cache = {}

# "fp32": plain fp32 Gram (LOW_HIGH, 4 HW passes per chunk pair)
# "fp16x3": h/l fp16 split, 6 single-cycle passes (h.h + h.l + l.h), ~1e-8
#           systematic error (PE computes fp16 subnormals exactly; verified)
MODE = "fp16x3"


def _build_nc(nt=NT, mode=None):
    import concourse.bacc as bacc
    import concourse.mybir as mybir
    from concourse.bass import ts
    from concourse.tile import TileContext

    if mode is None:
        mode = MODE
    f32 = mybir.dt.float32
    f16 = mybir.dt.float16
    u16 = mybir.dt.uint16

    nc = bacc.Bacc("TRN2")

    xin = nc.dram_tensor("xin", [C, N], f32, kind="ExternalInput")
    idx_out = nc.dram_tensor("idx8", [RBLK, 16], u16, kind="ExternalOutput")
    val_out = nc.dram_tensor("val8", [RBLK, 16], f32, kind="ExternalOutput")
    nrm_dram = nc.dram_tensor("nrm_scratch", [N], f32, kind="Internal")
    rn_dram = nc.dram_tensor("rn_scratch", [N], f32, kind="Internal")

    onesk_d = nc.inline_tensor(np.ones((128, 1), np.float32), name="onesk")
    eye_d = nc.inline_tensor(np.eye(128, dtype=np.float32) * NEG, name="eyeneg")

    DCH = 2048  # input DMA chunk

    with TileContext(nc) as tc:
        with (
            tc.tile_pool(name="consts", bufs=1) as cpool,
            tc.tile_pool(name="xpool", bufs=1) as xpool,
            tc.tile_pool(name="spool", bufs=3) as spool,
            tc.tile_pool(name="rpool", bufs=3) as rpool,
            tc.tile_pool(name="gpool", bufs=2) as gpool,
            tc.tile_pool(name="vpool", bufs=3) as vpool,
            tc.tile_pool(name="npsum", bufs=2, space="PSUM") as npsum,
            tc.tile_pool(name="gpsum", bufs=6, space="PSUM") as gpsum,
        ):
            ck = cpool.tile([128, 1], f32)
            nc.sync.dma_start(ck, onesk_d[:, :])
            eye = cpool.tile([128, 128], f32)
            nc.sync.dma_start(eye, eye_d[:, :])

            if mode == "fp32":
                # x in [C, N] layout: channels 0..127 in xA, 128..191 in xB
                # (rows 64..127 of xB zeroed for K=128 zero-padded matmuls).
                xA = xpool.tile([128, N], f32)
                xB = xpool.tile([128, N], f32)
                nc.gpsimd.memset(xB[64:128, :], 0.0)
                for dc in range(N // DCH):
                    dsl = ts(dc, DCH)
                    nc.sync.dma_start(xA[:, dsl], xin[0:128, dsl])
                    nc.sync.dma_start(xB[0:64, dsl], xin[128:192, dsl])

                nrm = cpool.tile([1, N], f32)
                for cc in range(NCHUNK):
                    sl = ts(cc, CHUNK)
                    sqA = spool.tile([128, CHUNK], f32)
                    nc.scalar.square(sqA, xA[:, sl])
                    sqB = spool.tile([128, CHUNK], f32)
                    nc.scalar.square(sqB, xB[:, sl])
                    nps = npsum.tile([1, CHUNK], f32)
                    nc.tensor.matmul(nps, ck, sqA, start=True, stop=False)
                    nc.tensor.matmul(nps, ck, sqB, start=False, stop=True)
                    nc.scalar.sqrt(nrm[:, sl], nps)
                nc.sync.dma_start(nrm_dram[None, :], nrm)

                # reciprocal in [128, 64] layout (DVE divide is per-lane; a
                # [1, N] reciprocal would run on one lane)
                nrmT = cpool.tile([128, N // 128], f32)
                nc.sync.dma_start(nrmT, nrm_dram[:].rearrange("(p f) -> p f", p=128))
                rnT = cpool.tile([128, N // 128], f32)
                nc.vector.reciprocal(rnT, nrmT)
                nc.sync.dma_start(rn_dram[:].rearrange("(p f) -> p f", p=128), rnT)

            if mode == "fp32":
                # normalize x in place: x *= (1/norm) broadcast over C.
                # 1/norm row is partition-broadcast by DMA (step-0 AP).
                for cc in range(NCHUNK):
                    sl = ts(cc, CHUNK)
                    rnb = rpool.tile([128, CHUNK], f32)
                    nc.sync.dma_start(
                        rnb, rn_dram[None, ts(cc, CHUNK)].to_broadcast([128, CHUNK])
                    )
                    nc.vector.tensor_mul(xA[:, sl], xA[:, sl], rnb)
                    nc.gpsimd.tensor_mul(xB[0:64, sl], xB[0:64, sl], rnb[0:64, :])

                for t in range(nt):
                    tsl = ts(t, 128)
                    g = gpool.tile([128, N], f32)
                    for cc in range(NCHUNK):
                        sl = ts(cc, CHUNK)
                        ps = gpsum.tile([128, CHUNK], f32)
                        nc.tensor.matmul(
                            ps, xA[:, tsl], xA[:, sl], start=True, stop=False
                        )
                        nc.tensor.matmul(
                            ps, xB[:, tsl], xB[:, sl], start=False, stop=True
                        )
                        nc.scalar.copy(g[:, sl], ps)
                    # knock out self-match diagonal (query p == column 128t+p)
                    nc.vector.tensor_add(g[:, tsl], g[:, tsl], eye)
                    v16 = vpool.tile([128, 16], f32)
                    i16 = vpool.tile([128, 16], u16)
                    H = N // 2
                    nc.vector.max(out=v16[:, 0:8], in_=g[:, 0:H])
                    nc.vector.max_index(i16[:, 0:8], v16[:, 0:8], g[:, 0:H])
                    nc.vector.max(out=v16[:, 8:16], in_=g[:, H:N])
                    nc.vector.max_index(i16[:, 8:16], v16[:, 8:16], g[:, H:N])
                    nc.sync.dma_start(idx_out[tsl, :], i16)
                    nc.sync.dma_start(val_out[tsl, :], v16)
            else:
                # fp16 split of the normalized points: xn = h + l/32 + O(2^-24)
                #   h  = fp16(xn)          l5 = fp16((xn - h) * 32)
                #   h5 = fp16(h / 32)
                # Gram accumulates h.h + h.(l/32*32) terms with exactly
                # cancelling power-of-two scales:
                #   h[t] x h  +  h5[t] x l5  +  l5[t] x h5
                hA = xpool.tile([128, N], f16)
                hBd = xpool.tile([128, N], f16)  # h_B duplicated in BOTH halves
                l5A = xpool.tile([128, N], f16)
                l5Bz = xpool.tile([128, N], f16)  # l5_B rows 0-63, zeros hi
                # composite weights W23B = [h_B ; l_B]: one K=128 pass against
                # moving hBd computes hh_B + lh_B together (5 Gram passes).
                # hl_B pairs w2Bz = hBd/32 with moving l5Bz (zero hi rows, so
                # the hi weights are inert).
                w2A = xpool.tile([128, RBLK], f16)
                w3A = xpool.tile([128, RBLK], f16)
                W23B = xpool.tile([128, RBLK], f16)
                w2Bz = xpool.tile([128, RBLK], f16)
                nc.gpsimd.memset(l5Bz[64:128, :], 0.0)

                # phase1 (norms) -> reciprocal -> build, pipelined in column
                # quarters so the build overlaps later quarters' norms.
                nrmT = cpool.tile([128, N // 128], f32)
                rnT = cpool.tile([128, N // 128], f32)
                BCH = 1024
                for cc in range(N // BCH):
                    sl = ts(cc, BCH)
                    xa = spool.tile([128, BCH], f32, tag="xa")
                    nc.sync.dma_start(xa, xin[0:128, sl])
                    xb = spool.tile([128, BCH], f32, tag="xb")
                    nc.gpsimd.memset(xb[64:128, :], 0.0)
                    nc.sync.dma_start(xb[0:64, :], xin[128:192, sl])
                    sqa = rpool.tile([128, BCH], f32, tag="rnb")
                    nc.vector.tensor_mul(sqa, xa, xa)
                    sqb = rpool.tile([128, BCH], f32, tag="rnb")
                    nc.vector.tensor_mul(sqb, xb, xb)
                    # fold the 64 B-channel squares into the A rows so one
                    # K=128 ones-matmul covers all 192 channels
                    nc.vector.tensor_add(sqa[0:64, :], sqa[0:64, :], sqb[0:64, :])
                    for hh in range(BCH // CHUNK):
                        hsl = slice(hh * CHUNK, (hh + 1) * CHUNK)
                        nps = npsum.tile([1, CHUNK], f32)
                        nc.tensor.matmul(nps, ck, sqa[:, hsl], start=True, stop=True)
                        nrmc = spool.tile([1, CHUNK], f32, tag="nrmc")
                        nc.scalar.sqrt(nrmc, nps)
                        nc.sync.dma_start(
                            nrm_dram[None, ts(cc * (BCH // CHUNK) + hh, CHUNK)],
                            nrmc,
                        )
                    if cc % 2 == 1:
                        # reciprocal for the finished 2048-col quarter
                        q = cc // 2
                        psl = slice(32 * q, 32 * (q + 1))
                        nc.sync.dma_start(
                            nrmT[psl, :],
                            nrm_dram[ts(q, 2048)].rearrange("(p f) -> p f", p=32),
                        )
                        nc.vector.reciprocal(rnT[psl, :], nrmT[psl, :])
                        nc.sync.dma_start(
                            rn_dram[ts(q, 2048)].rearrange("(p f) -> p f", p=32),
                            rnT[psl, :],
                        )
                if True:
                    for cc in range(N // BCH):
                        sl = ts(cc, BCH)
                        xa = spool.tile([128, BCH], f32, tag="xa")
                        nc.sync.dma_start(xa, xin[0:128, sl])
                        # B channels loaded into BOTH halves (the hi copy
                        # feeds the composite UB/WB tensors)
                        xb = spool.tile([128, BCH], f32, tag="xb")
                        nc.sync.dma_start(xb[0:64, :], xin[128:192, sl])
                        nc.sync.dma_start(xb[64:128, :], xin[128:192, sl])
                        rnb = rpool.tile([128, BCH], f32)
                        nc.sync.dma_start(
                            rnb, rn_dram[None, ts(cc, BCH)].to_broadcast([128, BCH])
                        )
                        nc.vector.tensor_mul(xa, xa, rnb)  # xa = xn (A half)
                        nc.vector.tensor_mul(xb, xb, rnb)  # xn_B, both halves
                        nc.scalar.copy(hA[:, sl], xa)  # cast to fp16 (ACT)
                        nc.scalar.copy(hBd[:, sl], xb)  # h_B dup, one full cast
                        nc.vector.tensor_sub(xa, xa, hA[:, sl])  # xa = xn - h
                        nc.vector.tensor_sub(
                            xb[0:64, :], xb[0:64, :], hBd[0:64, sl]
                        )
                        nc.scalar.mul(l5A[:, sl], xa, 32.0)
                        nc.scalar.mul(l5Bz[0:64, sl], xb[0:64, :], 32.0)
                        if (cc + 1) * BCH <= RBLK:
                            # w3_B = l_B plain (subnormal fp16 computes
                            # exactly on the PE), query columns only
                            nc.vector.tensor_sub(
                                xb[64:128, :], xb[64:128, :], hBd[64:128, sl]
                            )
                            nc.scalar.copy(W23B[64:128, ts(cc, BCH)], xb[64:128, :])
                        if cc == 1:
                            # weight-side scaled copies for the query columns
                            # (ready as soon as build chunks 0-1 land --
                            # issuing here lets the Gram's w-passes start
                            # ~6 build-chunks earlier):
                            #   w2 = h[:, :RBLK]/32 (vs moving l5 = l*32)
                            #   w3 = l[:, :RBLK] plain (vs moving h)
                            nc.vector.tensor_scalar_mul(w2A, hA[:, 0:RBLK], 0.03125)
                            nc.vector.tensor_scalar_mul(w3A, l5A[:, 0:RBLK], 0.03125)
                            nc.vector.tensor_copy(W23B[0:64, :], hBd[0:64, 0:RBLK])
                            nc.vector.tensor_scalar_mul(w2Bz, hBd[:, 0:RBLK], 0.03125)

                for t in range(nt):
                    tsl = ts(t, 128)
                    g = gpool.tile([128, N], f32)
                    for cc in range(NCHUNK):
                        sl = ts(cc, CHUNK)
                        ps = gpsum.tile([128, CHUNK], f32)
                        nc.tensor.matmul(
                            ps, hA[:, tsl], hA[:, sl], start=True, stop=False
                        )
                        nc.tensor.matmul(
                            ps, W23B[:, tsl], hBd[:, sl], start=False, stop=False
                        )
                        nc.tensor.matmul(
                            ps, w2A[:, tsl], l5A[:, sl], start=False, stop=False
                        )
                        nc.tensor.matmul(
                            ps, w3A[:, tsl], hA[:, sl], start=False, stop=False
                        )
                        nc.tensor.matmul(
                            ps, w2Bz[:, tsl], l5Bz[:, sl], start=False, stop=True
                        )
                        nc.scalar.copy(g[:, sl], ps)
                    nc.gpsimd.tensor_add(g[:, tsl], g[:, tsl], eye)
                    # top-8 per column half; host merges the 16 candidates
                    # by (-value, index) == jax top_k tie order. Half 1 can
                    # scan while the half-2 matmuls still run.
                    v16 = vpool.tile([128, 16], f32)
                    i16 = vpool.tile([128, 16], u16)
                    H = N // 2
                    nc.vector.max(out=v16[:, 0:8], in_=g[:, 0:H])
                    nc.vector.max_index(i16[:, 0:8], v16[:, 0:8], g[:, 0:H])
                    nc.vector.max(out=v16[:, 8:16], in_=g[:, H:N])
                    nc.vector.max_index(i16[:, 8:16], v16[:, 8:16], g[:, H:N])
                    nc.sync.dma_start(idx_out[tsl, :], i16)
                    nc.sync.dma_start(val_out[tsl, :], v16)

    nc.compile()
    return nc


def _get_nc():
    if "nc" not in _cache:
        _cache["nc"] = _build_nc()
    return _cache["nc"]


def shard_inputs(x):
    """x: [B, C, N, 1] -> list of 8 per-core input maps (rotated columns)."""
    xs = np.ascontiguousarray(np.asarray(x, dtype=np.float32).reshape(B, C, N))
    in_maps = []
    for c in range(NCORES):
        b, r = divmod(c, 4)
        s = r * RBLK
        xb = xs[b]
        rot = np.ascontiguousarray(np.roll(xb, -s, axis=1)) if s else xb
        in_maps.append({"xin": rot})
    return in_maps


def assemble(results):
    """results: 8 dicts with 'idx8' [RBLK, 16] u16 + 'val8' [RBLK, 16] f32.

    Each row holds the top-8 of each column half; merge by (-value,
    candidate position). Candidate positions are ordered so that stable
    sort reproduces jax.lax.top_k tie behavior (ascending index on equal
    values: within a half find_index8 assigns ascending indices, and
    half 1's indices all precede half 2's).
    """
    nn = np.empty((B, N, 9), np.int32)
    for c in range(NCORES):
        b, r = divmod(c, 4)
        s = r * RBLK
        i16 = results[c]["idx8"].astype(np.int64)
        v16 = results[c]["val8"]
        cand = i16
        cand[:, 8:] += N // 2
        order = np.argsort(-v16, axis=1, kind="stable")[:, :8]
        top8 = np.take_along_axis(cand, order, axis=1)
        nn[b, s : s + RBLK, 1:9] = (top8 + s) % N
        nn[b, s : s + RBLK, 0] = np.arange(s, s + RBLK)
    center = np.broadcast_to(np.arange(N, dtype=np.int32)[None, :, None], (B, N, 9))
    return np.ascontiguousarray(np.stack([nn, center], axis=0).astype(np.int32))


def kernel(x, _trace=False, **trace_kwargs):
    from concourse.bass_utils import run_bass_kernel_spmd

    nc = _get_nc()
    in_maps = shard_inputs(x)
    res = run_bass_kernel_spmd(
        nc, in_maps, core_ids=list(range(NCORES)), trace=_trace, **trace_kwargs
    )
    _cache["last_results"] = res
    return assemble(res.results)



# revision 2
# speedup vs baseline: 1.3350x; 1.3350x over previous
"""KNN graph kernel (DenseDilatedKnnGraph) for Trainium2, 8 NeuronCores.

Problem: x [2, 192, 8192, 1] fp32 -> edge_index [2, 2, 8192, 9] int32.
reference: L2-normalize x along C, pairwise sq-dists over N, top-9 (k=9,
dilation=1) nearest neighbors (indices), stacked with center indices.

Strategy (v2, candidate-screen + exact host rerank):
  For normalized points, ranking by -dist == ranking by cosine Xn^T Xn.
  The device computes an fp16 SCREENING Gram (2 matmul passes per 512-col
  chunk: K=128 "A" channels + K=64 "B" channels) and reduces each query
  row's 8192 columns to a 512-slot column-max array (slot s = max over
  the 16 columns == s mod 512) via a DVE fp16 tensor_max fold tree (fp16
  tensor_tensor runs in 2x_1p DVE mode; MAX8/FIND_INDEX8 have no fast
  mode, so their scan area must be small). Top-8 slots of each 256-slot
  half (depth-8 slack per half makes fp16 rank/tie perturbations
  irrelevant) -> 16 slots -> 256 candidate columns per row. The host
  re-ranks the 256 candidates with exact f64 dot products of the fp32
  normalized points, reproducing the reference top-8 exactly.

  The self column (cos ~ 1) is knocked out with -20 on the diagonal
  before folding so it cannot crowd out a real slot; the host prepends
  the self index (reference rank-1 neighbor is always self).

Sharding: 8 cores = 2 batches x 4 query-row-blocks of 2048. Each core
gets the full batch slice with its columns ROTATED so its own query
block sits at columns 0..2047 (keeps the SPMD program identical across
cores). Host maps neighbor indices back by adding the rotation offset.

Engine budget per row-tile (128 rows x 8192 cols), measured rates:
  PE   32 matmuls x ~237ns                     = 7.6us
  ACT  4 drains [128,2048] PSUM->SBUF fp16     = 8.0us   <- bound
  DVE  folds 4.3us + eye 0.13 + scans 1.31     = 5.8us
"""

import numpy as np

B = 2
C = 192
N = 8192
NCORES = 8
RBLK = N // 4  # 2048 query rows per core
NT = RBLK // 128  # 16 row tiles per core
NSLOT = 512  # column-max slots per row
NCOLS_PER_SLOT = N // NSLOT  # 16
NEG = -20.0

_cache = {}


def _build_nc(nt=NT):
    import concourse.bacc as bacc
    import concourse.mybir as mybir
    from concourse.bass import ts
    from concourse.tile import TileContext

    f32 = mybir.dt.float32
    f16 = mybir.dt.float16
    u16 = mybir.dt.uint16

    nc = bacc.Bacc("TRN2")

    xin = nc.dram_tensor("xin", [C, N], f16, kind="ExternalInput")
    idx_out = nc.dram_tensor("idx8", [RBLK, 16], u16, kind="ExternalOutput")
    val_out = nc.dram_tensor("val8", [RBLK, 16], f16, kind="ExternalOutput")

    eye_d = nc.inline_tensor(
        np.eye(128, dtype=np.float16) * np.float16(NEG), name="eyeneg"
    )

    with TileContext(nc) as tc:
        with (
            tc.tile_pool(name="consts", bufs=1) as cpool,
            tc.tile_pool(name="xpool", bufs=1) as xpool,
            tc.tile_pool(name="spool", bufs=6) as spool,
            tc.tile_pool(name="fpool", bufs=4) as fpool,
            tc.tile_pool(name="vpool", bufs=3) as vpool,
            tc.tile_pool(name="gpsum", bufs=2, space="PSUM") as gpsum,
        ):
            eye = cpool.tile([128, 128], f16)
            nc.sync.dma_start(eye, eye_d[:, :])

            # fp16 normalized points, prepared on host. A: channels 0..127,
            # B: channels 128..191 (K=64 second matmul pass).
            hA = xpool.tile([128, N], f16)
            hB = xpool.tile([64, N], f16)
            DCH = 1024
            for dc in range(N // DCH):
                dsl = ts(dc, DCH)
                nc.sync.dma_start(hA[:, dsl], xin[0:128, dsl])
                nc.sync.dma_start(hB[:, dsl], xin[128:192, dsl])

            for t in range(nt):
                tsl = ts(t, 128)
                S = []
                for q in range(4):
                    ps = gpsum.tile([128, 2048], f32)
                    for cch in range(4):
                        msl = slice(q * 2048 + cch * 512, q * 2048 + (cch + 1) * 512)
                        psl = slice(cch * 512, (cch + 1) * 512)
                        nc.tensor.matmul(
                            ps[:, psl], hA[:, tsl], hA[:, msl], start=True, stop=False
                        )
                        nc.tensor.matmul(
                            ps[:, psl], hB[:, tsl], hB[:, msl], start=False, stop=True
                        )
                    sq = spool.tile([128, 2048], f16, tag="s")
                    nc.scalar.copy(sq, ps)  # ACT drain + fp32->fp16 cast
                    S.append(sq)
                # knock out self-match (query p == rotated column 128t+p,
                # always inside quarter 0)
                esl = slice(128 * t, 128 * t + 128)
                nc.vector.tensor_add(S[0][:, esl], S[0][:, esl], eye)
                # fp16 fold tree -> [128, 512] slot maxima
                t01 = fpool.tile([128, 2048], f16, tag="t01")
                nc.vector.tensor_max(t01, S[0], S[1])
                t23 = fpool.tile([128, 2048], f16, tag="t23")
                nc.vector.tensor_max(t23, S[2], S[3])
                tq = fpool.tile([128, 2048], f16, tag="tq")
                nc.vector.tensor_max(tq, t01, t23)
                u = fpool.tile([128, 1024], f16, tag="u")
                nc.vector.tensor_max(u, tq[:, 0:1024], tq[:, 1024:2048])
                v = fpool.tile([128, NSLOT], f16, tag="v")
                nc.vector.tensor_max(v, u[:, 0:512], u[:, 512:1024])
                # top-8 slots per 256-slot half
                v16 = vpool.tile([128, 16], f16)
                i16 = vpool.tile([128, 16], u16)
                H = NSLOT // 2
                nc.vector.max(out=v16[:, 0:8], in_=v[:, 0:H])
                nc.vector.max_index(i16[:, 0:8], v16[:, 0:8], v[:, 0:H])
                nc.vector.max(out=v16[:, 8:16], in_=v[:, H:NSLOT])
                nc.vector.max_index(i16[:, 8:16], v16[:, 8:16], v[:, H:NSLOT])
                nc.sync.dma_start(idx_out[tsl, :], i16)
                nc.sync.dma_start(val_out[tsl, :], v16)

    nc.compile()
    return nc


def _get_nc():
    if "nc" not in _cache:
        _cache["nc"] = _build_nc()
    return _cache["nc"]


def _host_prep(x):
    """Normalize along C in fp32 (reference semantics), cast fp16."""
    xs = np.ascontiguousarray(np.asarray(x, dtype=np.float32).reshape(B, C, N))
    nrm = np.sqrt((xs * xs).sum(axis=1, keepdims=True))
    xn = xs / np.maximum(nrm, 1e-12)  # [B, C, N] f32
    h16 = xn.astype(np.float16)
    return xn, h16


def shard_inputs(h16):
    """h16: [B, C, N] f16 -> 8 per-core input maps (rotated columns)."""
    in_maps = []
    for c in range(NCORES):
        b, r = divmod(c, 4)
        s = r * RBLK
        hb = h16[b]
        rot = np.ascontiguousarray(np.roll(hb, -s, axis=1)) if s else hb
        in_maps.append({"xin": rot})
    return in_maps


def assemble(results, xn):
    """Exact rerank of the 256 screened candidates per row.

    results: 8 dicts with 'idx8' [RBLK, 16] u16 (cols 0:8 = top slots of
    half 0, cols 8:16 = half 1; slot in [0, 256) within its half).
    Candidate columns of slot s: s + 512*k, k=0..15 (rotated space).
    Rerank by exact f64 distance of the fp32 normalized points; ties by
    smaller index == jax.lax.top_k order.
    """
    nn = np.empty((B, N, 9), np.int32)
    kcols = (np.arange(NCOLS_PER_SLOT, dtype=np.int64) * NSLOT)[None, None, :]
    half_off = (np.arange(16, dtype=np.int64)[None, :] // 8) * (NSLOT // 2)
    sq = [(xn[b].astype(np.float64) ** 2).sum(axis=0) for b in range(B)]
    xbT = [np.ascontiguousarray(xn[b].T.astype(np.float64)) for b in range(B)]
    for c in range(NCORES):
        b, r = divmod(c, 4)
        s = r * RBLK
        slots = results[c]["idx8"].astype(np.int64) + half_off  # [RBLK, 16]
        cand = ((slots[:, :, None] + kcols) + s) % N  # [RBLK, 16, 16] global
        cand = cand.reshape(RBLK, 256)
        rows = np.arange(s, s + RBLK)
        qv = xbT[b][rows]  # [RBLK, C]
        vecs = xbT[b][cand]  # [RBLK, 256, C]
        inner = np.matmul(vecs, qv[:, :, None])[:, :, 0]  # [RBLK, 256]
        d = sq[b][rows][:, None] + sq[b][cand] - 2.0 * inner
        d[cand == rows[:, None]] = np.inf  # self handled separately
        order = np.lexsort((cand, d), axis=-1)[:, :8]
        nn[b, rows, 0] = rows
        nn[b, rows, 1:] = np.take_along_axis(cand, order, axis=1)
    center = np.broadcast_to(np.arange(N, dtype=np.int32)[None, :, None], (B, N, 9))
    return np.ascontiguousarray(np.stack([nn, center], axis=0).astype(np.int32))


def kernel(x, _trace=False, **trace_kwargs):
    from concourse.bass_utils import run_bass_kernel_spmd

    nc = _get_nc()
    xn, h16 = _host_prep(x)
    in_maps = shard_inputs(h16)
    res = run_bass_kernel_spmd(
        nc, in_maps, core_ids=list(range(NCORES)), trace=_trace, **trace_kwargs
    )
    _cache["last_results"] = res
    return assemble(res.results, xn)


# revision 4
# speedup vs baseline: 2.4978x; 1.8710x over previous
"""KNN graph kernel (DenseDilatedKnnGraph) for Trainium2, 8 NeuronCores.

Problem: x [2, 192, 8192, 1] fp32 -> edge_index [2, 2, 8192, 9] int32.
reference: L2-normalize x along C, pairwise sq-dists over N, top-9 (k=9,
dilation=1) nearest neighbors (indices), stacked with center indices.

Strategy (candidate-screen + exact host rerank):
  For normalized points, ranking by -dist == ranking by cosine Xn^T Xn.
  The device computes an fp16 SCREENING Gram (2 K=128 matmul passes per
  512-col chunk; B channels zero-padded to K=128 so the PE tile config
  never changes) and reduces each query row's 8192 columns to a 512-slot
  column-max array via a DVE fp16 tensor_max fold tree (fp16
  tensor_tensor runs at 2x_1p; MAX8/FIND_INDEX8 have no fast mode, so
  their scan area must be small). Top-8 slots of each 128-slot quarter
  -> 32 slots -> 512 candidate columns per row; the host re-ranks
  candidates exactly (f32 screen + f64 top-16 refine) against the fp32
  normalized points, reproducing the reference top-8.

  Slot geometry: the host PERMUTES moving columns P(c) = (c%16)*512 +
  c//16, so slot s (= permuted col mod 512) covers the 16 CONTIGUOUS
  original columns [16s, 16s+16). The data is diagonally correlated
  (neighbors cluster at col ~ row), so contiguous slots absorb clusters
  of near neighbors into one slot instead of crowding many.

  The self column (cos ~ 1) is knocked out with -20 via 4 constant
  [128,4,8] masks (its permuted position is row-dependent) before
  folding; the host prepends the self index (reference rank-1 neighbor
  is always self).

Sharding: 8 cores = 2 batches x 4 query-row-blocks of 2048. Each core
gets the full batch slice with columns ROTATED so its query block sits
at rotated columns 0..2047. Host maps indices back via the offset.

Per row-tile (128 rows x 8192 cols) engine budget, measured rates:
  PE   32 matmuls x ~215ns                      = 6.9us
  ACT  4 drains [128,2048] PSUM->SBUF fp16      = 8.0us   <- bound
  DVE  folds 4.3 + masks 0.4 + scans 1.6        = 6.3us
"""

import numpy as np

B = 2
C = 192
N = 8192
NCORES = 8
RBLK = N // 4  # 2048 query rows per core
NT = RBLK // 128  # 16 row tiles per core
NSLOT = 512
SLOTW = N // NSLOT  # 16 contiguous original columns per slot
NEG = -20.0

_cache = {}


def _self_masks():
    """mask[q][p, i, d] = NEG iff the self column of query row p lands at
    in-quarter position i*512 + 8t + d of quarter q (t enters via the AP
    column offset; the mask data is t-independent)."""
    masks = []
    for q in range(4):
        m = np.zeros((128, 4, 8), np.float16)
        for p in range(128):
            i = p % 16 - 4 * q
            if 0 <= i < 4:
                m[p, i, p // 16] = np.float16(NEG)
        masks.append(m.reshape(128, 32))
    return masks


def _build_nc(nt=NT):
    import concourse.bacc as bacc
    import concourse.mybir as mybir
    from concourse.bass import ts
    from concourse.tile import TileContext

    f32 = mybir.dt.float32
    f16 = mybir.dt.float16
    u16 = mybir.dt.uint16

    nc = bacc.Bacc("TRN2")

    # moving columns: rotated + permuted; query columns: rotated only
    xm = nc.dram_tensor("xm", [C, N], f16, kind="ExternalInput")
    xq = nc.dram_tensor("xq", [C, RBLK], f16, kind="ExternalInput")
    idx_out = nc.dram_tensor("idx8", [RBLK, 32], u16, kind="ExternalOutput")
    val_out = nc.dram_tensor("val8", [RBLK, 32], f16, kind="ExternalOutput")

    mask_d = [
        nc.inline_tensor(m, name=f"selfmask{q}") for q, m in enumerate(_self_masks())
    ]

    with TileContext(nc) as tc:
        with (
            tc.tile_pool(name="consts", bufs=1) as cpool,
            tc.tile_pool(name="xpool", bufs=1) as xpool,
            tc.tile_pool(name="spool", bufs=6) as spool,
            tc.tile_pool(name="fpool", bufs=4) as fpool,
            tc.tile_pool(name="vpool", bufs=3) as vpool,
            tc.tile_pool(name="gpsum", bufs=2, space="PSUM") as gpsum,
        ):
            masks = []
            for q in range(4):
                mk = cpool.tile([128, 32], f16, tag=f"mask{q}")
                nc.sync.dma_start(mk, mask_d[q][:, :])
                masks.append(mk.rearrange("p (i d) -> p i d", i=4))

            # fp16 normalized points (host-prepared). A: channels 0..127;
            # B: channels 128..191 in rows 0..63, rows 64..127 zeroed so
            # both passes are K=128 (the PE tile config must not change
            # between matmuls or it drops to the mid pstate).
            hmA = xpool.tile([128, N], f16)
            hmB = xpool.tile([128, N], f16)
            hqA = xpool.tile([128, RBLK], f16)
            hqB = xpool.tile([128, RBLK], f16)
            nc.gpsimd.memset(hmB[64:128, :], 0.0)
            nc.gpsimd.memset(hqB[64:128, :], 0.0)
            nc.sync.dma_start(hqA, xq[0:128, :])
            nc.sync.dma_start(hqB[0:64, :], xq[128:192, :])
            DCH = 1024
            for dc in range(N // DCH):
                dsl = ts(dc, DCH)
                nc.sync.dma_start(hmA[:, dsl], xm[0:128, dsl])
                nc.sync.dma_start(hmB[0:64, dsl], xm[128:192, dsl])

            for t in range(nt):
                tsl = ts(t, 128)
                S = []
                for q in range(4):
                    ps = gpsum.tile([128, 2048], f32)
                    for cch in range(4):
                        msl = slice(q * 2048 + cch * 512, q * 2048 + (cch + 1) * 512)
                        psl = slice(cch * 512, (cch + 1) * 512)
                        nc.tensor.matmul(
                            ps[:, psl], hqA[:, tsl], hmA[:, msl], start=True, stop=False
                        )
                        nc.tensor.matmul(
                            ps[:, psl], hqB[:, tsl], hmB[:, msl], start=False, stop=True
                        )
                    sq = spool.tile([128, 2048], f16, tag="s")
                    nc.scalar.copy(sq, ps)  # ACT drain + fp32->fp16 cast
                    # knock out the self column (row-dependent position)
                    sv = sq.rearrange("p (i f) -> p i f", i=4)[:, :, 8 * t : 8 * t + 8]
                    nc.vector.tensor_add(sv, sv, masks[q])
                    S.append(sq)
                # fp16 fold tree -> [128, 512] slot maxima
                t01 = fpool.tile([128, 2048], f16, tag="t01")
                nc.vector.tensor_max(t01, S[0], S[1])
                t23 = fpool.tile([128, 2048], f16, tag="t23")
                nc.vector.tensor_max(t23, S[2], S[3])
                tq = fpool.tile([128, 2048], f16, tag="tq")
                nc.vector.tensor_max(tq, t01, t23)
                u = fpool.tile([128, 1024], f16, tag="u")
                nc.vector.tensor_max(u, tq[:, 0:1024], tq[:, 1024:2048])
                v = fpool.tile([128, NSLOT], f16, tag="v")
                nc.vector.tensor_max(v, u[:, 0:512], u[:, 512:1024])
                # top-8 slots per 128-slot quarter (depth-8 slack per
                # quarter makes fp16 rank/tie perturbation irrelevant)
                v32 = vpool.tile([128, 32], f16)
                i32 = vpool.tile([128, 32], u16)
                for sq4 in range(4):
                    osl = slice(8 * sq4, 8 * sq4 + 8)
                    isl = slice(128 * sq4, 128 * (sq4 + 1))
                    nc.vector.max(out=v32[:, osl], in_=v[:, isl])
                    nc.vector.max_index(i32[:, osl], v32[:, osl], v[:, isl])
                nc.sync.dma_start(idx_out[tsl, :], i32)
                nc.sync.dma_start(val_out[tsl, :], v32)

    nc.compile()
    return nc


def _get_nc():
    if "nc" not in _cache:
        _cache["nc"] = _build_nc()
    return _cache["nc"]


def _host_prep(x):
    """Normalize along C in fp32 (reference semantics), cast fp16."""
    xs = np.ascontiguousarray(np.asarray(x, dtype=np.float32).reshape(B, C, N))
    nrm = np.sqrt((xs * xs).sum(axis=1, keepdims=True))
    xn = xs / np.maximum(nrm, 1e-12)  # [B, C, N] f32
    h16 = xn.astype(np.float16)
    return xn, h16


_PERM = None


def _perm():
    global _PERM
    if _PERM is None:
        c = np.arange(N)
        _PERM = np.empty(N, np.int64)
        _PERM[(c % SLOTW) * NSLOT + c // SLOTW] = c  # P(c) = (c%16)*512 + c//16
    return _PERM


def shard_inputs(h16):
    """h16: [B, C, N] f16 -> 8 per-core input maps."""
    perm = _perm()
    in_maps = []
    for c in range(NCORES):
        b, r = divmod(c, 4)
        s = r * RBLK
        rot = np.roll(h16[b], -s, axis=1) if s else h16[b]
        in_maps.append(
            {
                "xm": np.ascontiguousarray(rot[:, perm]),
                "xq": np.ascontiguousarray(rot[:, :RBLK]),
            }
        )
    return in_maps


def assemble(results, xn):
    """Exact rerank of the 512 screened candidates per row.

    results: 8 dicts with 'idx8' [RBLK, 32] u16; cols [8q:8q+8] hold the
    top slot indices (in [0,128)) of slot-quarter q. Slot s covers the 16
    contiguous rotated columns [16s, 16s+16). Rank by f32 distance, then
    refine the top 16 in f64; ties by smaller index == jax top_k order.
    """
    nn = np.empty((B, N, 9), np.int32)
    qoff = np.repeat(np.arange(4, dtype=np.int64) * 128, 8)[None, :]  # [1, 32]
    wcols = np.arange(SLOTW, dtype=np.int64)[None, None, :]
    sq64 = [(xn[b].astype(np.float64) ** 2).sum(axis=0) for b in range(B)]
    xbT = [np.ascontiguousarray(xn[b].T) for b in range(B)]  # [N, C] f32
    xbT64 = [a.astype(np.float64) for a in xbT]
    for c in range(NCORES):
        b, r = divmod(c, 4)
        s = r * RBLK
        slots = results[c]["idx8"].astype(np.int64) + qoff  # [RBLK, 32]
        cand = (slots[:, :, None] * SLOTW + wcols + s) % N  # [RBLK, 32, 16]
        cand = cand.reshape(RBLK, 32 * SLOTW)
        rows = np.arange(s, s + RBLK)
        # f32 screen over all 512 candidates
        qv = xbT[b][rows]  # [RBLK, C] f32
        vecs = xbT[b][cand]  # [RBLK, 512, C] f32
        inner = np.matmul(vecs, qv[:, :, None])[:, :, 0]
        d32 = -2.0 * inner + (xbT[b][cand] ** 2).sum(-1)
        d32[cand == rows[:, None]] = np.inf
        part = np.argpartition(d32, 16, axis=1)[:, :16]
        cand16 = np.take_along_axis(cand, part, axis=1)  # [RBLK, 16]
        # f64 exact rerank of the survivors
        qv64 = xbT64[b][rows]
        vecs64 = xbT64[b][cand16]
        inner64 = np.matmul(vecs64, qv64[:, :, None])[:, :, 0]
        d = sq64[b][rows][:, None] + sq64[b][cand16] - 2.0 * inner64
        d[cand16 == rows[:, None]] = np.inf
        order = np.lexsort((cand16, d), axis=-1)[:, :8]
        nn[b, rows, 0] = rows
        nn[b, rows, 1:] = np.take_along_axis(cand16, order, axis=1)
    center = np.broadcast_to(np.arange(N, dtype=np.int32)[None, :, None], (B, N, 9))
    return np.ascontiguousarray(np.stack([nn, center], axis=0).astype(np.int32))


def kernel(x, _trace=False, **trace_kwargs):
    from concourse.bass_utils import run_bass_kernel_spmd

    nc = _get_nc()
    xn, h16 = _host_prep(x)
    in_maps = shard_inputs(h16)
    res = run_bass_kernel_spmd(
        nc, in_maps, core_ids=list(range(NCORES)), trace=_trace, **trace_kwargs
    )
    _cache["last_results"] = res
    return assemble(res.results, xn)
